# revision 70
# baseline (speedup 1.0000x reference)
"""Ernie4 GQA attention layer as a Bass/Tile kernel for 8 TRN2 NeuronCores (v4).

Sharding: core c = 4*b + g handles batch b (of 2) and head-group g (of 4).
Each group owns 8 query heads + 1 kv head (GQA 32q/4kv, head_dim 128) and the
matching column slice of w_qkv / row slice of w_o. The o_proj partial sums are
reduced on the host (all-reduce equivalent).

v4 design: the two big GEMMs (qkv proj, o_proj) run as fp8e4 DoubleRow
matmuls (0.5 cycles/row, 2 contraction k-tiles per instruction = 4x bf16
FLOP rate) with 3-term error compensation:
    A@B ~= Aq@Bq + Ar@Bq + Aq@Br
where Aq = e4m3(A), Ar = e4m3(A - Aq) (unscaled residual), same for B.
Inputs are pre-scaled into e4m3's sweet spot (X*4, W*128, ctx*4) with the
scales folded into existing multiplicative knobs (psum->sbuf copy scale,
the ones matrix for softmax denominators, the o_proj output copy scale),
so compensation costs 0 extra elementwise ops for qkv and only 2 per ctx
tile for o_proj. Net: 48 DR matmuls per qkv tile vs 32 bf16 (0.75x cycles),
12 vs 16 for o_proj. Attention (scores, ctx) stays bf16: with K=128
contraction fp8 DR gains nothing at equal accuracy.

Inherited v3 structure: token-major p1 with interleaved per-head PE
transposes; RoPE via PSUM accumulation interleaved into attention(j=0);
attention j-outer/h-inner with transposed scores, exp on ACT, dual-chain
softmax denominators finished by a 0.25-scaled ones-matmul; o_proj work of
q-block j-1 interleaved into attention of q-block j (hb-major so the wo
DMA can stream in emission order).
"""
import sys

sys.path.insert(0, "/opt/trn_rl_repo")

import numpy as np

HIDDEN = 4096
N_Q_HEADS = 32
N_KV_HEADS = 4
HEAD_DIM = 128
ROPE_THETA = 500000.0
Q_SIZE = N_Q_HEADS * HEAD_DIM  # 4096
KV_SIZE = N_KV_HEADS * HEAD_DIM  # 512
B = 2
S = 2048
N_CORES = 8
N_GROUPS = 4
HEADS_PER_GROUP = N_Q_HEADS // N_GROUPS  # 8
GROUP_Q = HEADS_PER_GROUP * HEAD_DIM  # 1024
QKV_G = GROUP_Q + 2 * HEAD_DIM  # 1280 columns of qkv per group
SCALE = HEAD_DIM ** -0.5
NK = HIDDEN // 128  # 32 contraction k-tiles
NKT = S // 128  # 16 token/key tiles per sequence
NQB = S // 512  # 4 q-blocks
NHB = HIDDEN // 512  # 8 output-hidden blocks

# fp8 compensation scales: X*4, W*128 -> qkv psum = 512*qkv;
# ctx*4 (via 0.25-scaled ones), wo*128 -> o_proj psum = 512*out.
X_PRE = 4.0
W_PRE = 128.0
QKV_SCL = 1.0 / (X_PRE * W_PRE)  # 2^-9 on qkv psum->sbuf copies
OST_SCL = 1.0 / (4.0 * W_PRE)  # 2^-9 on o_proj psum->out copies

_COMPILED = None
LAST_EXEC_NS = None


def _build(phases=(1, 2, 3)):
    import concourse.mybir as mybir
    import concourse.tile as tile
    from concourse import bacc

    F32 = mybir.dt.float32
    F32R = mybir.dt.float32r
    BF16 = mybir.dt.bfloat16
    FP8 = mybir.dt.float8e4
    DR = mybir.MatmulPerfMode.DoubleRow

    nc = bacc.Bacc("TRN2", target_bir_lowering=False, debug=False, num_devices=N_CORES)

    # xq/xr: [tt, 128, NK, 128]; x*[tt, h, ko, t] = e4m3 hi/lo of
    # X_PRE * X[tt*128+t, ko*128+h] (contiguous 4KB per partition row)
    xq = nc.dram_tensor("xq", [NKT, 128, NK, 128], FP8, kind="ExternalInput").ap()
    xr = nc.dram_tensor("xr", [NKT, 128, NK, 128], FP8, kind="ExternalInput").ap()
    # wq/wr split per feature block (contiguous -> full-rate DMA descriptors):
    # w[s][fb]: [128, NK, fw]; hi/lo e4m3 of W_PRE*W^T[ko*128+p, c0+f]
    w_dram = {}
    for s in ("q", "r"):
        for fb, (c0, fw) in enumerate(((0, 256), (256, 512), (768, 512))):
            w_dram[(s, fb)] = nc.dram_tensor(
                f"w{s}{fb}", [128, NK, fw], FP8, kind="ExternalInput"
            ).ap()
    # woq/wor: [128, hk, 4096]; hi/lo e4m3 of W_PRE*w_o[o, g*1024+hk*128+p]
    woq = nc.dram_tensor("woq", [128, HEADS_PER_GROUP, HIDDEN], FP8,
                         kind="ExternalInput").ap()
    wor = nc.dram_tensor("wor", [128, HEADS_PER_GROUP, HIDDEN], FP8,
                         kind="ExternalInput").ap()
    # rope tables (bf16): cos2[2i]=cos2[2i+1]=cos; sin2[2i]=+sin, sin2[2i+1]=-sin
    cos2 = nc.dram_tensor("cos2", [HEAD_DIM, S], BF16, kind="ExternalInput").ap()
    sin2 = nc.dram_tensor("sin2", [HEAD_DIM, S], BF16, kind="ExternalInput").ap()
    swp = nc.dram_tensor("swp", [128, 128], BF16, kind="ExternalInput").ap()
    # ones scaled by 0.25 so rcp = 4/sum(p) and ctxb = 4*ctx (e4m3 sweet spot)
    ones = nc.dram_tensor("ones", [128, 128], F32R, kind="ExternalInput").ap()
    ident = nc.dram_tensor("ident", [128, 128], BF16, kind="ExternalInput").ap()
    # triangular edge mask: maskt[p, c] = 1 if c >= p else 0
    maskt = nc.dram_tensor("maskt", [128, 128], BF16, kind="ExternalInput").ap()
    out_part = nc.dram_tensor(
        "out_part", [NKT, NHB, 128, 512], BF16, kind="ExternalOutput"
    ).ap()

    FB = ((0, 256), (256, 512), (768, 512))  # (col0, width): kv | q0..3 | q4..7

    with tile.TileContext(nc) as tc:
        with (
            tc.tile_pool(name="consts", bufs=1) as cpool,
            tc.tile_pool(name="kvsb", bufs=1) as kvpool,
            tc.tile_pool(name="rqsb", bufs=1) as rqpool,
            tc.tile_pool(name="ropet", bufs=4) as ropet,
        ):
            swp_sb = cpool.tile([128, 128], BF16)
            ones_sb = cpool.tile([128, 128], F32R)
            id_sb = cpool.tile([128, 128], BF16)
            mask_sb = cpool.tile([128, 128], BF16)
            cos_sb = cpool.tile([128, S], BF16)
            sin_sb = cpool.tile([128, S], BF16)

            # persistent SBUF state
            kv_sb = [kvpool.tile([128, 256], BF16, name=f"kv{i}") for i in range(NKT)]
            rq = [rqpool.tile([128, S], BF16, name=f"rq{i}") for i in range(HEADS_PER_GROUP)]
            rk = rqpool.tile([128, S], BF16)

            def rope_chunk(t, c, pspool, pstag, dve_add=False, copy_dve=False):
                # t[:, cs] = swp@(t*sin') + I@(t*cos), psum-accumulated
                cs = slice(c * 512, (c + 1) * 512)
                m_sin = ropet.tile([128, 512], BF16, tag="msin", name="msin")
                nc.vector.tensor_mul(m_sin, t[:, cs], sin_sb[:, cs])
                m_cos = ropet.tile([128, 512], BF16, tag="mcos", name="mcos")
                nc.vector.tensor_mul(m_cos, t[:, cs], cos_sb[:, cs])
                ps = pspool.tile([128, 512], F32, tag=pstag, name="ropeps")
                if dve_add:
                    # phase-1 form: DVE (idle there) does the final add
                    nc.tensor.matmul(ps, swp_sb, m_sin, start=True, stop=True)
                    nc.vector.tensor_add(t[:, cs], ps, m_cos)
                    return
                nc.tensor.matmul(ps, swp_sb, m_sin, start=True, stop=False)
                nc.tensor.matmul(ps, id_sb, m_cos, start=False, stop=True)
                if copy_dve:
                    nc.vector.tensor_copy(t[:, cs], ps)
                else:
                    nc.scalar.copy(t[:, cs], ps)

            def rope_inplace(t, pspool, pstag, dve_add=False):
                for c in range(S // 512):
                    rope_chunk(t, c, pspool, pstag, dve_add=dve_add)

            # ---------------- phase 1: qkv + interleaved transposes ---------
            with (
                tc.tile_pool(name="w", bufs=1) as wpool,
                tc.tile_pool(name="xq", bufs=7) as xqpool,
                tc.tile_pool(name="xr", bufs=7) as xrpool,
                tc.tile_pool(name="qs", bufs=3) as qspool,
                tc.tile_pool(name="p1ps", bufs=3, space="PSUM") as p1ps,
                tc.tile_pool(name="tp", bufs=4, space="PSUM") as tpps,
            ):
                w_tiles = {}
                x_tiles = {}

                def emit_x(tt):
                    # X rides the second HWDGE queue (ACT) so it fair-shares
                    # the DMA engines with the W stream on SP instead of
                    # queueing behind it
                    xq_t = xqpool.tile([128, NK, 128], FP8, tag="xq", name="xqt")
                    nc.scalar.dma_start(xq_t, xq[tt])
                    xr_t = xrpool.tile([128, NK, 128], FP8, tag="xr", name="xrt")
                    nc.scalar.dma_start(xr_t, xr[tt])
                    x_tiles[tt] = (xq_t, xr_t)

                if 1 in phases:
                    # wq(fb0) + x0 + wr(fb0) first: the first p1 block's psum
                    # group needs all three terms' operands before it closes
                    for s in ("q", "r"):
                        for fb, (c0, fw) in enumerate(FB):
                            for kc in range(4):
                                wt = wpool.tile([128, 8, fw], FP8, name=f"w{s}{fb}_{kc}")
                                w_tiles[(s, fb, kc)] = wt

                    def wdma(s, fb, kc):
                        nc.sync.dma_start(
                            w_tiles[(s, fb, kc)],
                            w_dram[(s, fb)][:, kc * 8:(kc + 1) * 8, :],
                        )

                    # first W chunk in two halves so the PE starts sooner
                    nc.sync.dma_start(
                        w_tiles[("q", 0, 0)][:, :4, :], w_dram[("q", 0)][:, 0:4, :]
                    )
                    # x0 in two halves so the first kv matmuls start sooner
                    xq_t0 = xqpool.tile([128, NK, 128], FP8, tag="xq", name="xqt0")
                    nc.scalar.dma_start(xq_t0[:, :8], xq[0][:, :8])
                    nc.sync.dma_start(
                        w_tiles[("q", 0, 0)][:, 4:, :], w_dram[("q", 0)][:, 4:8, :]
                    )
                    nc.scalar.dma_start(xq_t0[:, 8:16], xq[0][:, 8:16])
                    wdma("q", 0, 1)
                    nc.scalar.dma_start(xq_t0[:, 16:], xq[0][:, 16:])
                    wdma("q", 0, 2), wdma("q", 0, 3)
                    xr_t0 = xrpool.tile([128, NK, 128], FP8, tag="xr", name="xrt0")
                    nc.scalar.dma_start(xr_t0, xr[0])
                    x_tiles[0] = (xq_t0, xr_t0)
                    for kc in range(4):
                        wdma("r", 0, kc)
                    nc.sync.dma_start(id_sb, ident)
                    emit_x(1)
                    emit_x(2)
                    emit_x(3)
                    for kc in range(4):
                        wdma("q", 1, kc)
                    for kc in range(4):
                        wdma("r", 1, kc)
                    for kc in range(4):
                        wdma("q", 2, kc)
                    for kc in range(4):
                        wdma("r", 2, kc)
                    # remaining consts trickle in behind the p1-critical DMAs
                    nc.sync.dma_start(swp_sb, swp)
                    nc.sync.dma_start(ones_sb, ones)
                    nc.sync.dma_start(mask_sb, maskt)
                    nc.sync.dma_start(cos_sb, cos2)
                    nc.sync.dma_start(sin_sb, sin2)

                def transpose_to(src):
                    tps = tpps.tile([128, 128], BF16, tag="tp", name="tps")
                    nc.tensor.transpose(tps, src, id_sb)
                    return tps

                pend_q = []  # deferred q transposes: (qs_tile, hh_base, tt)

                def p1_block(tt, fb):
                    c0, fw = FB[fb]
                    xq_t, xr_t = x_tiles[tt]
                    ps = p1ps.tile([128, 512], F32, tag="p1", name="p1t")
                    n_mm = 3 * (NK // 2)
                    mi = 0
                    # term order AqBq, ArBq, AqBr: xr arrives before wr via DMA
                    for a_t, w_s in ((xq_t, "q"), (xr_t, "q"), (xq_t, "r")):
                        for i in range(NK // 2):
                            kc, m = divmod(i, 4)
                            nc.tensor.matmul(
                                ps[:, :fw],
                                a_t[:, 2 * i:2 * i + 2, :],
                                w_tiles[(w_s, fb, kc)][:, 2 * m:2 * m + 2, :],
                                start=(mi == 0),
                                stop=(mi == n_mm - 1),
                                perf_mode=DR,
                            )
                            mi += 1
                    if fb == 0:
                        nc.vector.tensor_scalar_mul(kv_sb[tt], ps[:, :256], QKV_SCL)
                    else:
                        qs = qspool.tile([128, 512], BF16, tag="qs", name="qst")
                        nc.vector.tensor_scalar_mul(qs, ps[:, :fw], QKV_SCL)
                        pend_q.append((qs, (fb - 1) * 4, tt))
                    if fb == 1:
                        # k transpose emitted while later matmuls fill PE
                        tps = transpose_to(kv_sb[tt][:, 0:128])
                        nc.scalar.copy(rk[:, tt * 128:(tt + 1) * 128], tps)
                    # drain one pending q-transpose batch per block,
                    # lagging behind the DVE copies
                    if len(pend_q) > 1:
                        qs_t, hh0, qtt = pend_q.pop(0)
                        for hh in range(4):
                            h = hh0 + hh
                            tps = transpose_to(qs_t[:, hh * 128:(hh + 1) * 128])
                            nc.scalar.copy(
                                rq[h][:, qtt * 128:(qtt + 1) * 128], tps
                            )

                if 1 in phases:
                    # staggered quarter-major feature-block order: small first
                    # groups (2 token tiles) keep the early X demand under the
                    # DMA rate, later groups of 4 amortize; each group runs
                    # fb0..fb2 before the next group's X is needed
                    rope_chunk_ok = 2 in phases
                    groups = [(0, 4), (4, 8), (8, 12), (12, 16)]
                    rk_roped = 0
                    done_fb1 = 0
                    for gi, (g0, g1) in enumerate(groups):
                        if gi == 1:
                            # quarter 1 is X-DMA-gated: an fb-major fb0 pass
                            # consumes 4 X pairs in ~10us (>360GB/s). Pair
                            # (fb0,fb1) per token tile so each pair's first
                            # use is ~7.7us apart; fb2 trails (its W is
                            # resident by then)
                            order = [(tt, fb) for tt in range(g0, g1)
                                     for fb in (0, 1)]
                            order += [(tt, 2) for tt in range(g0, g1)]
                        else:
                            order = [(tt, fb) for fb in range(3)
                                     for tt in range(g0, g1)]
                        for tt, fb in order:
                            p1_block(tt, fb)
                            # prefetch next group's X at fb2: the reused
                            # buffer's last reader (x(tt-3)'s fb2 block)
                            # has already executed, so the DMA never waits
                            if fb == 2 and tt + 4 < NKT:
                                emit_x(tt + 4)
                        done_fb1 = g1
                        if rope_chunk_ok:
                            # rope rk and rq[0] per-quarter so the p1->attention
                            # transition has no serial rope chain left
                            while (rk_roped + 1) * 4 <= done_fb1:
                                rope_chunk(rk, rk_roped, p1ps, "p1", dve_add=True)
                                rope_chunk(rq[0], rk_roped, p1ps, "p1", dve_add=True)
                                rk_roped += 1
                for qs_t, hh0, qtt in pend_q:
                    for hh in range(4):
                        h = hh0 + hh
                        tps = transpose_to(qs_t[:, hh * 128:(hh + 1) * 128])
                        nc.scalar.copy(rq[h][:, qtt * 128:(qtt + 1) * 128], tps)

            # ---------------- phase 2/3: rope + attention + o_proj ----------
            with (
                tc.tile_pool(name="wo", bufs=1) as wopool,
                tc.tile_pool(name="pt", bufs=8) as ptpool,
                tc.tile_pool(name="pacc", bufs=4) as paccpool,
                tc.tile_pool(name="rcp", bufs=2) as rcppool,
                tc.tile_pool(name="ctxsb", bufs=2) as ctxsbpool,
                tc.tile_pool(name="ctxq8", bufs=2) as ctxq8pool,
                tc.tile_pool(name="ctxr8", bufs=2) as ctxr8pool,
                tc.tile_pool(name="ost", bufs=4) as ostpool,
                tc.tile_pool(name="scps", bufs=4, space="PSUM") as scps,
                tc.tile_pool(name="ctxps", bufs=2, space="PSUM") as ctxps,
                tc.tile_pool(name="opps", bufs=2, space="PSUM") as opps,
            ):
                woq_sb = wopool.tile([128, HEADS_PER_GROUP, HIDDEN], FP8)
                wor_sb = wopool.tile([128, HEADS_PER_GROUP, HIDDEN], FP8)
                if 3 in phases:
                    # hb-sliced in o_proj emission order so the first slices
                    # land before the first OpEmitter items run
                    for hb in range(NHB):
                        nc.sync.dma_start(
                            woq_sb[:, :, hb * 512:(hb + 1) * 512],
                            woq[:, :, hb * 512:(hb + 1) * 512],
                        )
                        nc.sync.dma_start(
                            wor_sb[:, :, hb * 512:(hb + 1) * 512],
                            wor[:, :, hb * 512:(hb + 1) * 512],
                        )

                # o_proj work items for q-block j, emitted interleaved during
                # attention of q-block j+1 (fills PE while ACT/Pool run).
                # hb-major so wo streams in; per item 12 fp8 DR matmuls
                # (4 head-pairs x 3 compensation terms).
                class OpEmitter:
                    def __init__(self, j, ctx_q8, ctx_r8):
                        self.items = [
                            (tl, hb)
                            for hb in range(NHB)
                            for tl in range(4)
                        ] if (3 in phases) else []
                        self.j = j
                        self.cq = ctx_q8
                        self.cr = ctx_r8
                        self.pos = 0

                    def emit(self, n):
                        # n counts DR-matmul triples (one head-pair, 3 terms)
                        for _ in range(n):
                            if self.pos >= 4 * len(self.items):
                                return
                            item, hp = divmod(self.pos, 4)
                            tl, hb = self.items[item]
                            ts = slice(tl * 128, (tl + 1) * 128)
                            hs = slice(2 * hp, 2 * hp + 2)
                            os_ = slice(hb * 512, (hb + 1) * 512)
                            if hp == 0:
                                self.ps = opps.tile([128, 512], F32, tag="op", name="opps")
                            for a_t, w_t in (
                                (self.cq, woq_sb),
                                (self.cq, wor_sb),
                                (self.cr, woq_sb),
                            ):
                                nc.tensor.matmul(
                                    self.ps,
                                    a_t[:, hs, ts],
                                    w_t[:, hs, os_],
                                    start=(hp == 0 and a_t is self.cq and w_t is woq_sb),
                                    stop=(hp == 3 and a_t is self.cr),
                                    perf_mode=DR,
                                )
                            if hp == 3:
                                ost = ostpool.tile([128, 512], BF16, tag="ost", name="ost")
                                if item % 2 == 0:
                                    nc.vector.tensor_scalar_mul(ost, self.ps, OST_SCL)
                                else:
                                    nc.scalar.activation(
                                        ost, self.ps,
                                        mybir.ActivationFunctionType.Copy,
                                        scale=OST_SCL,
                                    )
                                nc.sync.dma_start(
                                    out_part[self.j * 4 + tl, hb], ost
                                )
                            self.pos += 1

                    def flush(self):
                        self.emit(4 * len(self.items) - self.pos)

                def finalize(fin, pool_merge=False):
                    pacc_a, pacc_b, ctx_ps, ctx_dst, ctx_q, ctx_r = fin
                    acc = pacc_a
                    if pacc_b is not None:
                        # at j=0 merge on Pool: keeps the DVE queue (rope muls)
                        # out of the PE ones-matmul's critical path; elsewhere
                        # Pool is loaded with pacc chains, DVE is better
                        eng = nc.gpsimd if pool_merge else nc.vector
                        eng.tensor_add(
                            pacc_a, pacc_a.bitcast(F32), pacc_b.bitcast(F32)
                        )
                    r_ps = scps.tile([128, 512], F32, tag="sc", name="rpst")
                    nc.tensor.matmul(r_ps, ones_sb, acc, start=True, stop=True)
                    rcp = rcppool.tile([128, 512], F32, tag="rcp", name="rcpt")
                    nc.vector.reciprocal(rcp, r_ps)
                    # ctxb = 4*ctx (ones are 0.25-scaled); fp8 split for o_proj
                    nc.vector.tensor_mul(ctx_dst, ctx_ps, rcp)
                    nc.gpsimd.tensor_copy(ctx_q, ctx_dst)
                    nc.vector.tensor_tensor(
                        ctx_r, ctx_dst, ctx_q, mybir.AluOpType.subtract
                    )

                prev_op = None
                fin = None
                for j in range(NQB if 2 in phases else 0):
                    nkt_j = 4 * (j + 1)
                    # diagonal tiles first (descending width), then full tiles
                    kt_order = list(range(4 * j, 4 * j + 4)) + list(range(4 * j))
                    ctx_sb_j = ctxsbpool.tile(
                        [128, HEADS_PER_GROUP, 512], BF16, tag="ctx", name="ctxsb"
                    )
                    ctx_q8_j = ctxq8pool.tile(
                        [128, HEADS_PER_GROUP, 512], FP8, tag="ctxq", name="ctxq8"
                    )
                    ctx_r8_j = ctxr8pool.tile(
                        [128, HEADS_PER_GROUP, 512], FP8, tag="ctxr", name="ctxr8"
                    )
                    op_budget = 0.0
                    # 128 DR-triples per j, paced over 8*nkt_j attention steps
                    op_step = (16.0 / nkt_j) if prev_op is not None else 0.0
                    for h in range(HEADS_PER_GROUP):
                        ctx_ps = ctxps.tile([128, 512], F32, tag="ctxp", name="ctxpt")
                        pacc_a = paccpool.tile([128, 512], F32R, tag="pacca", name="pacca")
                        pacc_b = None
                        b_init = False
                        flip = False

                        def sc_emit(i):
                            kt = kt_order[i]
                            di = kt - 4 * j
                            col0 = di * 128 if di >= 0 else 0
                            sc_ps = scps.tile([128, 512], F32, tag="sc", name="scpst")
                            nc.tensor.matmul(
                                sc_ps[:, col0:],
                                rk[:, kt * 128:(kt + 1) * 128],
                                rq[h][:, j * 512 + col0:(j + 1) * 512],
                                start=True,
                                stop=True,
                            )
                            return sc_ps, kt, col0

                        pend = [sc_emit(0)]
                        if nkt_j > 1:
                            pend.append(sc_emit(1))
                        if nkt_j > 2:
                            pend.append(sc_emit(2))
                        # at j=0 the previous head's finalize lands after the
                        # score prefill: the PE has 3 matmuls + rope in flight
                        # to hide the pacc-merge latency before the ones-matmul
                        if j == 0 and fin is not None:
                            finalize(fin, pool_merge=True)
                            fin = None
                        for i in range(nkt_j):
                            sc_ps, kt, col0 = pend.pop(0)
                            di = kt - 4 * j
                            pt = ptpool.tile([128, 512], BF16, tag="pt", name="ptt")
                            nc.scalar.activation(
                                pt[:, col0:], sc_ps[:, col0:],
                                mybir.ActivationFunctionType.Exp,
                                scale=SCALE,
                            )
                            if di >= 0:  # diagonal: mask triangular edge
                                nc.vector.tensor_mul(
                                    pt[:, col0:col0 + 128],
                                    pt[:, col0:col0 + 128],
                                    mask_sb,
                                )
                            # dual-chain denominator accumulation
                            if i == 0:
                                nc.gpsimd.tensor_copy(pacc_a, pt)
                            elif col0 == 0 and not b_init:
                                pacc_b = paccpool.tile(
                                    [128, 512], F32R, tag="paccb", name="paccb"
                                )
                                nc.vector.tensor_copy(pacc_b, pt)
                                b_init = True
                            elif (not b_init) or flip:
                                nc.gpsimd.tensor_add(
                                    pacc_a[:, col0:],
                                    pacc_a[:, col0:].bitcast(F32),
                                    pt[:, col0:],
                                )
                                flip = False
                            else:
                                nc.vector.tensor_add(
                                    pacc_b[:, col0:],
                                    pacc_b[:, col0:].bitcast(F32),
                                    pt[:, col0:],
                                )
                                flip = True
                            if prev_op is not None:
                                op_budget += op_step
                                n_emit = int(op_budget)
                                op_budget -= n_emit
                                prev_op.emit(n_emit)
                            nc.tensor.matmul(
                                ctx_ps[:, col0:],
                                kv_sb[kt][:, 128:256],
                                pt[:, col0:],
                                start=(i == 0),
                                stop=(i == nkt_j - 1),
                                skip_group_check=True,
                            )
                            if i + 3 < nkt_j:
                                pend.append(sc_emit(i + 3))
                            if i == 0 and j > 0 and fin is not None:
                                finalize(fin)
                                fin = None
                        fin = (pacc_a, pacc_b, ctx_ps, ctx_sb_j[:, h, :],
                               ctx_q8_j[:, h, :], ctx_r8_j[:, h, :])
                        # interleave rope of the next head into attention(j=0)
                        # (psum form: j=0 is latency-bound, PE needs the filler)
                        if j == 0 and h + 1 < HEADS_PER_GROUP:
                            rope_inplace(rq[h + 1], scps, "sc")
                    if prev_op is not None:
                        prev_op.flush()
                    prev_op = OpEmitter(j, ctx_q8_j, ctx_r8_j)
                if fin is not None:
                    finalize(fin)
                    fin = None
                if prev_op is not None:
                    prev_op.flush()

    nc.compile()
    return nc


def _host_inputs(positions, hidden_states, w_qkv, w_o):
    """Shard + fp8-split + lay out inputs for the 8 cores (c = 4*b + g)."""
    import ml_dtypes

    bf16 = ml_dtypes.bfloat16
    fp8 = ml_dtypes.float8_e4m3
    positions = np.asarray(positions)
    hidden_states = np.asarray(hidden_states, dtype=np.float32)
    w_qkv = np.asarray(w_qkv, dtype=np.float32)
    w_o = np.asarray(w_o, dtype=np.float32)

    def split8(a):
        hi = a.astype(fp8)
        lo = (a - hi.astype(np.float32)).astype(fp8)
        return hi, lo

    inv_freq = 1.0 / (ROPE_THETA ** (np.arange(0, HEAD_DIM, 2, dtype=np.float64) / HEAD_DIM))
    ang = positions.astype(np.float64)[None, :] * inv_freq[:, None]  # [half, S]
    c = np.cos(ang).astype(np.float32)
    s = np.sin(ang).astype(np.float32)
    cos2 = np.empty((HEAD_DIM, S), dtype=np.float32)
    sin2 = np.empty((HEAD_DIM, S), dtype=np.float32)
    cos2[0::2] = c
    cos2[1::2] = c
    sin2[0::2] = s
    sin2[1::2] = -s

    swp = np.zeros((128, 128), dtype=np.float32)
    idx = np.arange(0, 128, 2)
    swp[idx, idx + 1] = 1.0
    swp[idx + 1, idx] = 1.0
    ones = np.full((128, 128), 0.25, dtype=np.float32)
    ident = np.eye(128, dtype=np.float32)
    maskt = (np.arange(128)[None, :] >= np.arange(128)[:, None]).astype(np.float32)

    xqs, xrs = [], []
    for b in range(B):
        xt_t = np.ascontiguousarray(
            (X_PRE * hidden_states[b]).reshape(NKT, 128, NK, 128).transpose(0, 3, 2, 1)
        )  # [tt, h, ko, t] f32
        hi, lo = split8(xt_t)
        xqs.append(hi)
        xrs.append(lo)

    wqs, wrs, woqs, wors = [], [], [], []
    for g in range(N_GROUPS):
        cols = np.concatenate([
            np.arange(Q_SIZE + g * HEAD_DIM, Q_SIZE + (g + 1) * HEAD_DIM),  # k
            np.arange(Q_SIZE + KV_SIZE + g * HEAD_DIM, Q_SIZE + KV_SIZE + (g + 1) * HEAD_DIM),  # v
            np.arange(g * GROUP_Q, (g + 1) * GROUP_Q),  # q0..q7
        ])
        wq_g = W_PRE * w_qkv[cols, :]  # [1280, 4096]
        wqkvt_t = np.ascontiguousarray(
            wq_g.T.reshape(NK, 128, QKV_G).transpose(1, 0, 2)
        )
        hi, lo = split8(wqkvt_t)
        wqs.append(hi)
        wrs.append(lo)  # each [128, NK, 1280]; sliced per fb below
        wot_full = W_PRE * w_o[:, g * GROUP_Q:(g + 1) * GROUP_Q].T  # [1024, 4096]
        wot_t = np.ascontiguousarray(
            wot_full.reshape(HEADS_PER_GROUP, 128, HIDDEN).transpose(1, 0, 2)
        )
        hi, lo = split8(wot_t)
        woqs.append(hi)
        wors.append(lo)

    FBH = ((0, 256), (256, 512), (768, 512))
    in_maps = []
    for c_id in range(N_CORES):
        b, g = divmod(c_id, N_GROUPS)
        wmap = {}
        for s, arr in (("q", wqs[g]), ("r", wrs[g])):
            for fb, (c0, fw) in enumerate(FBH):
                wmap[f"w{s}{fb}"] = np.ascontiguousarray(arr[:, :, c0:c0 + fw])
        in_maps.append({
            "xq": xqs[b],
            "xr": xrs[b],
            **wmap,
            "woq": woqs[g],
            "wor": wors[g],
            "cos2": cos2.astype(bf16),
            "sin2": sin2.astype(bf16),
            "swp": swp.astype(bf16),
            "ones": ones,
            "ident": ident.astype(bf16),
            "maskt": maskt.astype(bf16),
        })
    return in_maps


def kernel(positions, hidden_states, w_qkv, w_o):
    global _COMPILED, LAST_EXEC_NS
    from concourse import bass_utils

    if _COMPILED is None:
        _COMPILED = _build()
    nc = _COMPILED

    in_maps = _host_inputs(positions, hidden_states, w_qkv, w_o)
    res = bass_utils.run_bass_kernel_spmd(
        nc, in_maps, core_ids=list(range(N_CORES))
    )
    LAST_EXEC_NS = res.exec_time_ns

    out = np.zeros((B, S, HIDDEN), dtype=np.float32)
    for c_id in range(N_CORES):
        b = c_id // N_GROUPS
        part = res.results[c_id]["out_part"]  # [NKT, NHB, 128, 512] bf16
        out[b] += part.astype(np.float32).transpose(0, 2, 1, 3).reshape(S, HIDDEN)
    return out


# revision 71
# speedup vs baseline: 1.0003x; 1.0003x over previous
"""Ernie4 GQA attention layer as a Bass/Tile kernel for 8 TRN2 NeuronCores (v4).

Sharding: core c = 4*b + g handles batch b (of 2) and head-group g (of 4).
Each group owns 8 query heads + 1 kv head (GQA 32q/4kv, head_dim 128) and the
matching column slice of w_qkv / row slice of w_o. The o_proj partial sums are
reduced on the host (all-reduce equivalent).

v4 design: the two big GEMMs (qkv proj, o_proj) run as fp8e4 DoubleRow
matmuls (0.5 cycles/row, 2 contraction k-tiles per instruction = 4x bf16
FLOP rate) with 3-term error compensation:
    A@B ~= Aq@Bq + Ar@Bq + Aq@Br
where Aq = e4m3(A), Ar = e4m3(A - Aq) (unscaled residual), same for B.
Inputs are pre-scaled into e4m3's sweet spot (X*4, W*128, ctx*4) with the
scales folded into existing multiplicative knobs (psum->sbuf copy scale,
the ones matrix for softmax denominators, the o_proj output copy scale),
so compensation costs 0 extra elementwise ops for qkv and only 2 per ctx
tile for o_proj. Net: 48 DR matmuls per qkv tile vs 32 bf16 (0.75x cycles),
12 vs 16 for o_proj. Attention (scores, ctx) stays bf16: with K=128
contraction fp8 DR gains nothing at equal accuracy.

Inherited v3 structure: token-major p1 with interleaved per-head PE
transposes; RoPE via PSUM accumulation interleaved into attention(j=0);
attention j-outer/h-inner with transposed scores, exp on ACT, dual-chain
softmax denominators finished by a 0.25-scaled ones-matmul; o_proj work of
q-block j-1 interleaved into attention of q-block j (hb-major so the wo
DMA can stream in emission order).
"""
import sys

sys.path.insert(0, "/opt/trn_rl_repo")

import numpy as np

HIDDEN = 4096
N_Q_HEADS = 32
N_KV_HEADS = 4
HEAD_DIM = 128
ROPE_THETA = 500000.0
Q_SIZE = N_Q_HEADS * HEAD_DIM  # 4096
KV_SIZE = N_KV_HEADS * HEAD_DIM  # 512
B = 2
S = 2048
N_CORES = 8
N_GROUPS = 4
HEADS_PER_GROUP = N_Q_HEADS // N_GROUPS  # 8
GROUP_Q = HEADS_PER_GROUP * HEAD_DIM  # 1024
QKV_G = GROUP_Q + 2 * HEAD_DIM  # 1280 columns of qkv per group
SCALE = HEAD_DIM ** -0.5
NK = HIDDEN // 128  # 32 contraction k-tiles
NKT = S // 128  # 16 token/key tiles per sequence
NQB = S // 512  # 4 q-blocks
NHB = HIDDEN // 512  # 8 output-hidden blocks

# fp8 compensation scales: X*4, W*128 -> qkv psum = 512*qkv;
# ctx*4 (via 0.25-scaled ones), wo*128 -> o_proj psum = 512*out.
X_PRE = 4.0
W_PRE = 128.0
QKV_SCL = 1.0 / (X_PRE * W_PRE)  # 2^-9 on qkv psum->sbuf copies
OST_SCL = 1.0 / (4.0 * W_PRE)  # 2^-9 on o_proj psum->out copies

_COMPILED = None
LAST_EXEC_NS = None


def _build(phases=(1, 2, 3)):
    import concourse.mybir as mybir
    import concourse.tile as tile
    from concourse import bacc

    F32 = mybir.dt.float32
    F32R = mybir.dt.float32r
    BF16 = mybir.dt.bfloat16
    FP8 = mybir.dt.float8e4
    DR = mybir.MatmulPerfMode.DoubleRow

    nc = bacc.Bacc("TRN2", target_bir_lowering=False, debug=False, num_devices=N_CORES)

    # xq/xr: [tt, 128, NK, 128]; x*[tt, h, ko, t] = e4m3 hi/lo of
    # X_PRE * X[tt*128+t, ko*128+h] (contiguous 4KB per partition row)
    xq = nc.dram_tensor("xq", [NKT, 128, NK, 128], FP8, kind="ExternalInput").ap()
    xr = nc.dram_tensor("xr", [NKT, 128, NK, 128], FP8, kind="ExternalInput").ap()
    # wq/wr split per feature block (contiguous -> full-rate DMA descriptors):
    # w[s][fb]: [128, NK, fw]; hi/lo e4m3 of W_PRE*W^T[ko*128+p, c0+f]
    w_dram = {}
    for s in ("q", "r"):
        for fb, (c0, fw) in enumerate(((0, 256), (256, 512), (768, 512))):
            w_dram[(s, fb)] = nc.dram_tensor(
                f"w{s}{fb}", [128, NK, fw], FP8, kind="ExternalInput"
            ).ap()
    # woq/wor: [128, hk, 4096]; hi/lo e4m3 of W_PRE*w_o[o, g*1024+hk*128+p]
    woq = nc.dram_tensor("woq", [128, HEADS_PER_GROUP, HIDDEN], FP8,
                         kind="ExternalInput").ap()
    wor = nc.dram_tensor("wor", [128, HEADS_PER_GROUP, HIDDEN], FP8,
                         kind="ExternalInput").ap()
    # rope tables (bf16): cos2[2i]=cos2[2i+1]=cos; sin2[2i]=+sin, sin2[2i+1]=-sin
    cos2 = nc.dram_tensor("cos2", [HEAD_DIM, S], BF16, kind="ExternalInput").ap()
    sin2 = nc.dram_tensor("sin2", [HEAD_DIM, S], BF16, kind="ExternalInput").ap()
    swp = nc.dram_tensor("swp", [128, 128], BF16, kind="ExternalInput").ap()
    # ones scaled by 0.25 so rcp = 4/sum(p) and ctxb = 4*ctx (e4m3 sweet spot)
    ones = nc.dram_tensor("ones", [128, 128], F32R, kind="ExternalInput").ap()
    ident = nc.dram_tensor("ident", [128, 128], BF16, kind="ExternalInput").ap()
    # triangular edge mask: maskt[p, c] = 1 if c >= p else 0
    maskt = nc.dram_tensor("maskt", [128, 128], BF16, kind="ExternalInput").ap()
    out_part = nc.dram_tensor(
        "out_part", [NKT, NHB, 128, 512], BF16, kind="ExternalOutput"
    ).ap()

    FB = ((0, 256), (256, 512), (768, 512))  # (col0, width): kv | q0..3 | q4..7

    with tile.TileContext(nc) as tc:
        with (
            tc.tile_pool(name="consts", bufs=1) as cpool,
            tc.tile_pool(name="kvsb", bufs=1) as kvpool,
            tc.tile_pool(name="rqsb", bufs=1) as rqpool,
            tc.tile_pool(name="ropet", bufs=4) as ropet,
        ):
            swp_sb = cpool.tile([128, 128], BF16)
            ones_sb = cpool.tile([128, 128], F32R)
            id_sb = cpool.tile([128, 128], BF16)
            mask_sb = cpool.tile([128, 128], BF16)
            cos_sb = cpool.tile([128, S], BF16)
            sin_sb = cpool.tile([128, S], BF16)

            # persistent SBUF state
            kv_sb = [kvpool.tile([128, 256], BF16, name=f"kv{i}") for i in range(NKT)]
            rq = [rqpool.tile([128, S], BF16, name=f"rq{i}") for i in range(HEADS_PER_GROUP)]
            rk = rqpool.tile([128, S], BF16)

            def rope_chunk(t, c, pspool, pstag, dve_add=False, copy_dve=False):
                # t[:, cs] = swp@(t*sin') + I@(t*cos), psum-accumulated
                cs = slice(c * 512, (c + 1) * 512)
                m_sin = ropet.tile([128, 512], BF16, tag="msin", name="msin")
                nc.vector.tensor_mul(m_sin, t[:, cs], sin_sb[:, cs])
                m_cos = ropet.tile([128, 512], BF16, tag="mcos", name="mcos")
                nc.vector.tensor_mul(m_cos, t[:, cs], cos_sb[:, cs])
                ps = pspool.tile([128, 512], F32, tag=pstag, name="ropeps")
                if dve_add:
                    # phase-1 form: DVE (idle there) does the final add
                    nc.tensor.matmul(ps, swp_sb, m_sin, start=True, stop=True)
                    nc.vector.tensor_add(t[:, cs], ps, m_cos)
                    return
                nc.tensor.matmul(ps, swp_sb, m_sin, start=True, stop=False)
                nc.tensor.matmul(ps, id_sb, m_cos, start=False, stop=True)
                if copy_dve:
                    nc.vector.tensor_copy(t[:, cs], ps)
                else:
                    nc.scalar.copy(t[:, cs], ps)

            def rope_inplace(t, pspool, pstag, dve_add=False):
                for c in range(S // 512):
                    rope_chunk(t, c, pspool, pstag, dve_add=dve_add)

            # ---------------- phase 1: qkv + interleaved transposes ---------
            with (
                tc.tile_pool(name="w", bufs=1) as wpool,
                tc.tile_pool(name="xq", bufs=7) as xqpool,
                tc.tile_pool(name="xr", bufs=7) as xrpool,
                tc.tile_pool(name="qs", bufs=3) as qspool,
                tc.tile_pool(name="p1ps", bufs=3, space="PSUM") as p1ps,
                tc.tile_pool(name="tp", bufs=4, space="PSUM") as tpps,
            ):
                w_tiles = {}
                x_tiles = {}

                def emit_x(tt):
                    # X rides the second HWDGE queue (ACT) so it fair-shares
                    # the DMA engines with the W stream on SP instead of
                    # queueing behind it
                    xq_t = xqpool.tile([128, NK, 128], FP8, tag="xq", name="xqt")
                    nc.scalar.dma_start(xq_t, xq[tt])
                    xr_t = xrpool.tile([128, NK, 128], FP8, tag="xr", name="xrt")
                    nc.scalar.dma_start(xr_t, xr[tt])
                    x_tiles[tt] = (xq_t, xr_t)

                if 1 in phases:
                    # wq(fb0) + x0 + wr(fb0) first: the first p1 block's psum
                    # group needs all three terms' operands before it closes
                    for s in ("q", "r"):
                        for fb, (c0, fw) in enumerate(FB):
                            for kc in range(4):
                                wt = wpool.tile([128, 8, fw], FP8, name=f"w{s}{fb}_{kc}")
                                w_tiles[(s, fb, kc)] = wt

                    def wdma(s, fb, kc):
                        nc.sync.dma_start(
                            w_tiles[(s, fb, kc)],
                            w_dram[(s, fb)][:, kc * 8:(kc + 1) * 8, :],
                        )

                    # first W chunk in two halves so the PE starts sooner
                    nc.sync.dma_start(
                        w_tiles[("q", 0, 0)][:, :4, :], w_dram[("q", 0)][:, 0:4, :]
                    )
                    # x0 in two halves so the first kv matmuls start sooner
                    xq_t0 = xqpool.tile([128, NK, 128], FP8, tag="xq", name="xqt0")
                    nc.scalar.dma_start(xq_t0[:, :8], xq[0][:, :8])
                    nc.sync.dma_start(
                        w_tiles[("q", 0, 0)][:, 4:, :], w_dram[("q", 0)][:, 4:8, :]
                    )
                    nc.scalar.dma_start(xq_t0[:, 8:16], xq[0][:, 8:16])
                    wdma("q", 0, 1)
                    nc.scalar.dma_start(xq_t0[:, 16:], xq[0][:, 16:])
                    wdma("q", 0, 2), wdma("q", 0, 3)
                    xr_t0 = xrpool.tile([128, NK, 128], FP8, tag="xr", name="xrt0")
                    nc.scalar.dma_start(xr_t0, xr[0])
                    x_tiles[0] = (xq_t0, xr_t0)
                    for kc in range(4):
                        wdma("r", 0, kc)
                    nc.sync.dma_start(id_sb, ident)
                    emit_x(1)
                    emit_x(2)
                    emit_x(3)
                    for kc in range(4):
                        wdma("q", 1, kc)
                    for kc in range(4):
                        wdma("r", 1, kc)
                    for kc in range(4):
                        wdma("q", 2, kc)
                    for kc in range(4):
                        wdma("r", 2, kc)
                    # remaining consts trickle in behind the p1-critical DMAs
                    nc.sync.dma_start(swp_sb, swp)
                    nc.sync.dma_start(ones_sb, ones)
                    nc.sync.dma_start(mask_sb, maskt)
                    nc.sync.dma_start(cos_sb, cos2)
                    nc.sync.dma_start(sin_sb, sin2)

                def transpose_to(src):
                    tps = tpps.tile([128, 128], BF16, tag="tp", name="tps")
                    nc.tensor.transpose(tps, src, id_sb)
                    return tps

                pend_q = []  # deferred q transposes: (qs_tile, hh_base, tt)

                def p1_block(tt, fb):
                    c0, fw = FB[fb]
                    xq_t, xr_t = x_tiles[tt]
                    ps = p1ps.tile([128, 512], F32, tag="p1", name="p1t")
                    n_mm = 3 * (NK // 2)
                    mi = 0
                    # term order AqBq, ArBq, AqBr: xr arrives before wr via DMA
                    for a_t, w_s in ((xq_t, "q"), (xr_t, "q"), (xq_t, "r")):
                        for i in range(NK // 2):
                            kc, m = divmod(i, 4)
                            nc.tensor.matmul(
                                ps[:, :fw],
                                a_t[:, 2 * i:2 * i + 2, :],
                                w_tiles[(w_s, fb, kc)][:, 2 * m:2 * m + 2, :],
                                start=(mi == 0),
                                stop=(mi == n_mm - 1),
                                perf_mode=DR,
                            )
                            mi += 1
                    if fb == 0:
                        nc.vector.tensor_scalar_mul(kv_sb[tt], ps[:, :256], QKV_SCL)
                    else:
                        qs = qspool.tile([128, 512], BF16, tag="qs", name="qst")
                        nc.vector.tensor_scalar_mul(qs, ps[:, :fw], QKV_SCL)
                        pend_q.append((qs, (fb - 1) * 4, tt))
                    if fb == 1:
                        # k transpose emitted while later matmuls fill PE
                        tps = transpose_to(kv_sb[tt][:, 0:128])
                        nc.scalar.copy(rk[:, tt * 128:(tt + 1) * 128], tps)
                    # drain one pending q-transpose batch per block,
                    # lagging behind the DVE copies
                    if len(pend_q) > 1:
                        qs_t, hh0, qtt = pend_q.pop(0)
                        for hh in range(4):
                            h = hh0 + hh
                            tps = transpose_to(qs_t[:, hh * 128:(hh + 1) * 128])
                            nc.scalar.copy(
                                rq[h][:, qtt * 128:(qtt + 1) * 128], tps
                            )

                if 1 in phases:
                    # staggered quarter-major feature-block order: small first
                    # groups (2 token tiles) keep the early X demand under the
                    # DMA rate, later groups of 4 amortize; each group runs
                    # fb0..fb2 before the next group's X is needed
                    rope_chunk_ok = 2 in phases
                    groups = [(0, 4), (4, 8), (8, 12), (12, 16)]
                    rk_roped = 0
                    done_fb1 = 0
                    for g0, g1 in groups:
                        for fb in range(3):
                            for tt in range(g0, g1):
                                p1_block(tt, fb)
                                # prefetch next group's X at fb2: the reused
                                # buffer's last reader (x(tt-3)'s fb2 block)
                                # has already executed, so the DMA never waits
                                if fb == 2 and tt + 4 < NKT:
                                    emit_x(tt + 4)
                            if fb == 1:
                                done_fb1 = g1
                        if rope_chunk_ok:
                            # rope rk and rq[0] per-quarter so the p1->attention
                            # transition has no serial rope chain left
                            while (rk_roped + 1) * 4 <= done_fb1:
                                rope_chunk(rk, rk_roped, p1ps, "p1", dve_add=True)
                                rope_chunk(rq[0], rk_roped, p1ps, "p1", dve_add=True)
                                rk_roped += 1
                for qs_t, hh0, qtt in pend_q:
                    for hh in range(4):
                        h = hh0 + hh
                        tps = transpose_to(qs_t[:, hh * 128:(hh + 1) * 128])
                        nc.scalar.copy(rq[h][:, qtt * 128:(qtt + 1) * 128], tps)

            # ---------------- phase 2/3: rope + attention + o_proj ----------
            with (
                tc.tile_pool(name="wo", bufs=1) as wopool,
                tc.tile_pool(name="pt", bufs=8) as ptpool,
                tc.tile_pool(name="pacc", bufs=4) as paccpool,
                tc.tile_pool(name="rcp", bufs=2) as rcppool,
                tc.tile_pool(name="ctxsb", bufs=2) as ctxsbpool,
                tc.tile_pool(name="ctxq8", bufs=2) as ctxq8pool,
                tc.tile_pool(name="ctxr8", bufs=2) as ctxr8pool,
                tc.tile_pool(name="ost", bufs=4) as ostpool,
                tc.tile_pool(name="scps", bufs=4, space="PSUM") as scps,
                tc.tile_pool(name="ctxps", bufs=2, space="PSUM") as ctxps,
                tc.tile_pool(name="opps", bufs=2, space="PSUM") as opps,
            ):
                woq_sb = wopool.tile([128, HEADS_PER_GROUP, HIDDEN], FP8)
                wor_sb = wopool.tile([128, HEADS_PER_GROUP, HIDDEN], FP8)
                if 3 in phases:
                    # hb-sliced in o_proj emission order so the first slices
                    # land before the first OpEmitter items run
                    for hb in range(NHB):
                        nc.sync.dma_start(
                            woq_sb[:, :, hb * 512:(hb + 1) * 512],
                            woq[:, :, hb * 512:(hb + 1) * 512],
                        )
                        nc.sync.dma_start(
                            wor_sb[:, :, hb * 512:(hb + 1) * 512],
                            wor[:, :, hb * 512:(hb + 1) * 512],
                        )

                # o_proj work items for q-block j, emitted interleaved during
                # attention of q-block j+1 (fills PE while ACT/Pool run).
                # hb-major so wo streams in; per item 12 fp8 DR matmuls
                # (4 head-pairs x 3 compensation terms).
                class OpEmitter:
                    def __init__(self, j, ctx_q8, ctx_r8):
                        self.items = [
                            (tl, hb)
                            for hb in range(NHB)
                            for tl in range(4)
                        ] if (3 in phases) else []
                        self.j = j
                        self.cq = ctx_q8
                        self.cr = ctx_r8
                        self.pos = 0

                    def emit(self, n):
                        # n counts DR-matmul triples (one head-pair, 3 terms)
                        for _ in range(n):
                            if self.pos >= 4 * len(self.items):
                                return
                            item, hp = divmod(self.pos, 4)
                            tl, hb = self.items[item]
                            ts = slice(tl * 128, (tl + 1) * 128)
                            hs = slice(2 * hp, 2 * hp + 2)
                            os_ = slice(hb * 512, (hb + 1) * 512)
                            if hp == 0:
                                self.ps = opps.tile([128, 512], F32, tag="op", name="opps")
                            for a_t, w_t in (
                                (self.cq, woq_sb),
                                (self.cq, wor_sb),
                                (self.cr, woq_sb),
                            ):
                                nc.tensor.matmul(
                                    self.ps,
                                    a_t[:, hs, ts],
                                    w_t[:, hs, os_],
                                    start=(hp == 0 and a_t is self.cq and w_t is woq_sb),
                                    stop=(hp == 3 and a_t is self.cr),
                                    perf_mode=DR,
                                )
                            if hp == 3:
                                ost = ostpool.tile([128, 512], BF16, tag="ost", name="ost")
                                if item % 2 == 0:
                                    nc.vector.tensor_scalar_mul(ost, self.ps, OST_SCL)
                                else:
                                    nc.scalar.activation(
                                        ost, self.ps,
                                        mybir.ActivationFunctionType.Copy,
                                        scale=OST_SCL,
                                    )
                                nc.sync.dma_start(
                                    out_part[self.j * 4 + tl, hb], ost
                                )
                            self.pos += 1

                    def flush(self):
                        self.emit(4 * len(self.items) - self.pos)

                def finalize(fin, pool_merge=False):
                    pacc_a, pacc_b, ctx_ps, ctx_dst, ctx_q, ctx_r = fin
                    acc = pacc_a
                    if pacc_b is not None:
                        # at j=0 merge on Pool: keeps the DVE queue (rope muls)
                        # out of the PE ones-matmul's critical path; elsewhere
                        # Pool is loaded with pacc chains, DVE is better
                        eng = nc.gpsimd if pool_merge else nc.vector
                        eng.tensor_add(
                            pacc_a, pacc_a.bitcast(F32), pacc_b.bitcast(F32)
                        )
                    r_ps = scps.tile([128, 512], F32, tag="sc", name="rpst")
                    nc.tensor.matmul(r_ps, ones_sb, acc, start=True, stop=True)
                    rcp = rcppool.tile([128, 512], F32, tag="rcp", name="rcpt")
                    nc.vector.reciprocal(rcp, r_ps)
                    # ctxb = 4*ctx (ones are 0.25-scaled); fp8 split for o_proj
                    nc.vector.tensor_mul(ctx_dst, ctx_ps, rcp)
                    nc.gpsimd.tensor_copy(ctx_q, ctx_dst)
                    nc.vector.tensor_tensor(
                        ctx_r, ctx_dst, ctx_q, mybir.AluOpType.subtract
                    )

                prev_op = None
                fin = None
                for j in range(NQB if 2 in phases else 0):
                    nkt_j = 4 * (j + 1)
                    # diagonal tiles first (descending width), then full tiles
                    kt_order = list(range(4 * j, 4 * j + 4)) + list(range(4 * j))
                    ctx_sb_j = ctxsbpool.tile(
                        [128, HEADS_PER_GROUP, 512], BF16, tag="ctx", name="ctxsb"
                    )
                    ctx_q8_j = ctxq8pool.tile(
                        [128, HEADS_PER_GROUP, 512], FP8, tag="ctxq", name="ctxq8"
                    )
                    ctx_r8_j = ctxr8pool.tile(
                        [128, HEADS_PER_GROUP, 512], FP8, tag="ctxr", name="ctxr8"
                    )
                    op_budget = 0.0
                    # 128 DR-triples per j, paced over 8*nkt_j attention steps
                    op_step = (16.0 / nkt_j) if prev_op is not None else 0.0
                    for h in range(HEADS_PER_GROUP):
                        ctx_ps = ctxps.tile([128, 512], F32, tag="ctxp", name="ctxpt")
                        pacc_a = paccpool.tile([128, 512], F32R, tag="pacca", name="pacca")
                        pacc_b = None
                        b_init = False
                        flip = False

                        def sc_emit(i):
                            kt = kt_order[i]
                            di = kt - 4 * j
                            col0 = di * 128 if di >= 0 else 0
                            sc_ps = scps.tile([128, 512], F32, tag="sc", name="scpst")
                            nc.tensor.matmul(
                                sc_ps[:, col0:],
                                rk[:, kt * 128:(kt + 1) * 128],
                                rq[h][:, j * 512 + col0:(j + 1) * 512],
                                start=True,
                                stop=True,
                            )
                            return sc_ps, kt, col0

                        pend = [sc_emit(0)]
                        if nkt_j > 1:
                            pend.append(sc_emit(1))
                        if nkt_j > 2:
                            pend.append(sc_emit(2))
                        # at j=0 the previous head's finalize lands after the
                        # score prefill: the PE has 3 matmuls + rope in flight
                        # to hide the pacc-merge latency before the ones-matmul
                        if j == 0 and fin is not None:
                            finalize(fin, pool_merge=True)
                            fin = None
                        for i in range(nkt_j):
                            sc_ps, kt, col0 = pend.pop(0)
                            di = kt - 4 * j
                            pt = ptpool.tile([128, 512], BF16, tag="pt", name="ptt")
                            nc.scalar.activation(
                                pt[:, col0:], sc_ps[:, col0:],
                                mybir.ActivationFunctionType.Exp,
                                scale=SCALE,
                            )
                            if di >= 0:  # diagonal: mask triangular edge
                                nc.vector.tensor_mul(
                                    pt[:, col0:col0 + 128],
                                    pt[:, col0:col0 + 128],
                                    mask_sb,
                                )
                            # dual-chain denominator accumulation
                            if i == 0:
                                nc.gpsimd.tensor_copy(pacc_a, pt)
                            elif col0 == 0 and not b_init:
                                pacc_b = paccpool.tile(
                                    [128, 512], F32R, tag="paccb", name="paccb"
                                )
                                nc.vector.tensor_copy(pacc_b, pt)
                                b_init = True
                            elif (not b_init) or flip:
                                nc.gpsimd.tensor_add(
                                    pacc_a[:, col0:],
                                    pacc_a[:, col0:].bitcast(F32),
                                    pt[:, col0:],
                                )
                                flip = False
                            else:
                                nc.vector.tensor_add(
                                    pacc_b[:, col0:],
                                    pacc_b[:, col0:].bitcast(F32),
                                    pt[:, col0:],
                                )
                                flip = True
                            if prev_op is not None:
                                op_budget += op_step
                                n_emit = int(op_budget)
                                op_budget -= n_emit
                                prev_op.emit(n_emit)
                            nc.tensor.matmul(
                                ctx_ps[:, col0:],
                                kv_sb[kt][:, 128:256],
                                pt[:, col0:],
                                start=(i == 0),
                                stop=(i == nkt_j - 1),
                                skip_group_check=True,
                            )
                            if i + 3 < nkt_j:
                                pend.append(sc_emit(i + 3))
                            if i == 0 and j > 0 and fin is not None:
                                finalize(fin)
                                fin = None
                        fin = (pacc_a, pacc_b, ctx_ps, ctx_sb_j[:, h, :],
                               ctx_q8_j[:, h, :], ctx_r8_j[:, h, :])
                        # interleave rope of the next head into attention(j=0)
                        # (psum form: j=0 is latency-bound, PE needs the filler)
                        if j == 0 and h + 1 < HEADS_PER_GROUP:
                            rope_inplace(rq[h + 1], scps, "sc")
                    if prev_op is not None:
                        prev_op.flush()
                    prev_op = OpEmitter(j, ctx_q8_j, ctx_r8_j)
                if fin is not None:
                    finalize(fin)
                    fin = None
                if prev_op is not None:
                    prev_op.flush()

    nc.compile()
    return nc


def _host_inputs(positions, hidden_states, w_qkv, w_o):
    """Shard + fp8-split + lay out inputs for the 8 cores (c = 4*b + g)."""
    import ml_dtypes

    bf16 = ml_dtypes.bfloat16
    fp8 = ml_dtypes.float8_e4m3
    positions = np.asarray(positions)
    hidden_states = np.asarray(hidden_states, dtype=np.float32)
    w_qkv = np.asarray(w_qkv, dtype=np.float32)
    w_o = np.asarray(w_o, dtype=np.float32)

    def split8(a):
        hi = a.astype(fp8)
        lo = (a - hi.astype(np.float32)).astype(fp8)
        return hi, lo

    inv_freq = 1.0 / (ROPE_THETA ** (np.arange(0, HEAD_DIM, 2, dtype=np.float64) / HEAD_DIM))
    ang = positions.astype(np.float64)[None, :] * inv_freq[:, None]  # [half, S]
    c = np.cos(ang).astype(np.float32)
    s = np.sin(ang).astype(np.float32)
    cos2 = np.empty((HEAD_DIM, S), dtype=np.float32)
    sin2 = np.empty((HEAD_DIM, S), dtype=np.float32)
    cos2[0::2] = c
    cos2[1::2] = c
    sin2[0::2] = s
    sin2[1::2] = -s

    swp = np.zeros((128, 128), dtype=np.float32)
    idx = np.arange(0, 128, 2)
    swp[idx, idx + 1] = 1.0
    swp[idx + 1, idx] = 1.0
    ones = np.full((128, 128), 0.25, dtype=np.float32)
    ident = np.eye(128, dtype=np.float32)
    maskt = (np.arange(128)[None, :] >= np.arange(128)[:, None]).astype(np.float32)

    xqs, xrs = [], []
    for b in range(B):
        xt_t = np.ascontiguousarray(
            (X_PRE * hidden_states[b]).reshape(NKT, 128, NK, 128).transpose(0, 3, 2, 1)
        )  # [tt, h, ko, t] f32
        hi, lo = split8(xt_t)
        xqs.append(hi)
        xrs.append(lo)

    wqs, wrs, woqs, wors = [], [], [], []
    for g in range(N_GROUPS):
        cols = np.concatenate([
            np.arange(Q_SIZE + g * HEAD_DIM, Q_SIZE + (g + 1) * HEAD_DIM),  # k
            np.arange(Q_SIZE + KV_SIZE + g * HEAD_DIM, Q_SIZE + KV_SIZE + (g + 1) * HEAD_DIM),  # v
            np.arange(g * GROUP_Q, (g + 1) * GROUP_Q),  # q0..q7
        ])
        wq_g = W_PRE * w_qkv[cols, :]  # [1280, 4096]
        wqkvt_t = np.ascontiguousarray(
            wq_g.T.reshape(NK, 128, QKV_G).transpose(1, 0, 2)
        )
        hi, lo = split8(wqkvt_t)
        wqs.append(hi)
        wrs.append(lo)  # each [128, NK, 1280]; sliced per fb below
        wot_full = W_PRE * w_o[:, g * GROUP_Q:(g + 1) * GROUP_Q].T  # [1024, 4096]
        wot_t = np.ascontiguousarray(
            wot_full.reshape(HEADS_PER_GROUP, 128, HIDDEN).transpose(1, 0, 2)
        )
        hi, lo = split8(wot_t)
        woqs.append(hi)
        wors.append(lo)

    FBH = ((0, 256), (256, 512), (768, 512))
    in_maps = []
    for c_id in range(N_CORES):
        b, g = divmod(c_id, N_GROUPS)
        wmap = {}
        for s, arr in (("q", wqs[g]), ("r", wrs[g])):
            for fb, (c0, fw) in enumerate(FBH):
                wmap[f"w{s}{fb}"] = np.ascontiguousarray(arr[:, :, c0:c0 + fw])
        in_maps.append({
            "xq": xqs[b],
            "xr": xrs[b],
            **wmap,
            "woq": woqs[g],
            "wor": wors[g],
            "cos2": cos2.astype(bf16),
            "sin2": sin2.astype(bf16),
            "swp": swp.astype(bf16),
            "ones": ones,
            "ident": ident.astype(bf16),
            "maskt": maskt.astype(bf16),
        })
    return in_maps


def kernel(positions, hidden_states, w_qkv, w_o):
    global _COMPILED, LAST_EXEC_NS
    from concourse import bass_utils

    if _COMPILED is None:
        _COMPILED = _build()
    nc = _COMPILED

    in_maps = _host_inputs(positions, hidden_states, w_qkv, w_o)
    res = bass_utils.run_bass_kernel_spmd(
        nc, in_maps, core_ids=list(range(N_CORES))
    )
    LAST_EXEC_NS = res.exec_time_ns

    out = np.zeros((B, S, HIDDEN), dtype=np.float32)
    for c_id in range(N_CORES):
        b = c_id // N_GROUPS
        part = res.results[c_id]["out_part"]  # [NKT, NHB, 128, 512] bf16
        out[b] += part.astype(np.float32).transpose(0, 2, 1, 3).reshape(S, HIDDEN)
    return out
